# revision 10
# baseline (speedup 1.0000x reference)
"""ConvolutionalAttention (training branch) for Trainium2, 8 NeuronCores.

The module computes, per sample b:
    out[:, :32]  = conv13x13(x1, lk_filter) + depthwise3x3(x1, dyn_k[b])
    out[:, 32:]  = x2            (pass-through)
where dyn_k[b] comes from a tiny MLP (pool -> 1x1 -> GELU -> 1x1) on x1.

Key transformation: conv is linear in the filter, so the per-sample dynamic
depthwise 3x3 kernel is folded host-side into the center of a per-sample
13x13 dense filter.  The device then runs ONE dense 32->32 13x13 conv per
sample.  Data-parallel over batch: 2 samples per core.

Device mapping (per core, per sample):
  - conv as matmul with K = 128 = (4 row-shift replicas g) x (32 in-ch),
    M = 128 = (4 output rows dy) x (32 out-ch), rhs free dim N = 384 =
    two 4-row "quads" (8 output rows) x 192 useful columns each, read via
    an overlapped access pattern into the 204-wide padded rows.
  - 52 weight blocks (4 ky'-chunks x 13 kx shifts) accumulate in PSUM.
  - bf16 operands (fp32 PSUM accumulation): full PE rate (163ns per
    N=384 matmul), half the DMA traffic of tf32, rel err ~2.5e-3.
  - single 128-partition DMAs (the DRAM AP factors the partition dim as
    [row-shift 4] x [channel 32]) reach ~245GB/s vs ~100 for per-group
    loads; weights are host-transposed so each chunk is per-partition
    contiguous.
  - prologue: pairs {0,1} run before {2,3} on the first step so compute
    starts after ~60% of the first band has landed; a PE prewarm loop
    (garbage matmuls into a scratch PSUM bank) ramps the clock to full
    speed before real work arrives.
  - last step processes pairs singly so drains/output DMAs pipeline with
    the remaining matmuls instead of serializing at the end.
"""

import json

import ml_dtypes
import numpy as np

import concourse.bass as bass
import concourse.mybir as mybir
import concourse.tile as tile
from concourse.bass_utils import run_bass_kernel_spmd

# ---------------------------------------------------------------------------
# Problem constants (hardcoded; kernel.py must be self-contained)
B, C, H, W = 16, 64, 192, 192
PD, SK, LK = 32, 3, 13
PAD = LK // 2                      # 6
NCORES = 8
BLOC = B // NCORES                 # 2 samples per core
PADW = W + 2 * PAD                 # 204
PADH = H + 2 * PAD                 # 204
NJ, G, DY = 4, 4, 4                # ky' chunks, row-shift replicas, rows/quad
NKX = LK                           # 13 kx shifts
NBLK = NJ * NKX                    # 52 weight blocks per sample
BANDS = 6                          # 32 output rows per band
PAIRS = 4                          # quad-pairs per band (8 rows each)
SROWS = 41                         # X4 rows needed per band
NFREE = 2 * W                      # 384 matmul moving free dim (2 quads)
NWARM = 56                         # prewarm matmuls (bridge to data-ready)
F32 = mybir.dt.float32
BF16 = mybir.dt.bfloat16

# ---------------------------------------------------------------------------
# Workaround: the walrus_driver in this container rejects instructions with
# more than one sync-wait command.  Post-process the BIR JSON, moving excess
# waits onto single-wait NoOps inserted right before the offending
# instruction (same engine => executes first, semantics preserved).
_orig_to_json_bytes = bass.Bass.to_json_bytes


def _split_multi_waits(m):
    for f in m.get("functions", []):
        for blk in f.get("blocks", []):
            out = []
            changed = False
            for inst in blk.get("instructions", []):
                si = inst.get("sync_info")
                waits = (si or {}).get("on_wait") or []
                if len(waits) > 1:
                    changed = True
                    for k, wcond in enumerate(waits[:-1]):
                        out.append({
                            "debug": inst.get("debug"),
                            "engine": inst["engine"],
                            "ins": [], "outs": [],
                            "name": f"{inst['name']}.sw{k}",
                            "opcode": "NoOp",
                            "sync_info": {"on_update": [], "on_wait": [wcond]},
                            "text_hint": "split_wait",
                        })
                    si["on_wait"] = [waits[-1]]
                out.append(inst)
            if changed:
                blk["instructions"] = out
    return m


def _to_json_bytes_split(self, *a, **kw):
    data = _orig_to_json_bytes(self, *a, **kw)
    return json.dumps(_split_multi_waits(json.loads(data))).encode()


def _install_patch():
    if bass.Bass.to_json_bytes is not _to_json_bytes_split:
        bass.Bass.to_json_bytes = _to_json_bytes_split


# ---------------------------------------------------------------------------
# Device kernel


def _build_nc():
    _install_patch()
    nc = bass.Bass()
    xin = nc.declare_dram_parameter("xin", [BLOC, PD, PADH, PADW], BF16,
                                    isOutput=False)
    # host-transposed: [b, k(=g*32+ic), j, kx, m(=dy*32+oc)] so each j-chunk
    # is per-partition contiguous (13*128*2B = 3.3KB descriptors)
    wts = nc.declare_dram_parameter("wts", [BLOC, 128, NJ, NKX, 128], BF16,
                                    isOutput=False)
    yout = nc.declare_dram_parameter("yout", [BLOC, PD, H, W], F32,
                                     isOutput=True)
    xin_ap = xin.ap()
    yout_ap = yout.ap()

    with tile.TileContext(nc) as tc:
        with tc.tile_pool(name="wp", bufs=2) as wp, \
             tc.tile_pool(name="xp", bufs=3) as xp, \
             tc.tile_pool(name="pp", bufs=2, space="PSUM") as pp, \
             tc.tile_pool(name="op", bufs=4) as op:

            ENG = [nc.sync, nc.scalar]

            # PE prewarm: garbage matmuls into a scratch PSUM bank while the
            # prologue DMAs stream in, so HAM is at full clock (K=8/8) when
            # the real stream starts.  Reuses acc0's rotation slot.
            wi = xp.tile([128, NFREE], BF16, tag="wi")
            nc.vector.memset(wi[:], 0)
            warm = pp.tile([128, NFREE], F32, tag="acc0", name="warm")
            for i in range(NWARM):
                nc.tensor.matmul(warm[:], wi[:, :128], wi[:],
                                 start=(i == 0), stop=(i == NWARM - 1))

            def load_piece(x4, b, band, s0, s1):
                # per-g DMAs (one ascending address run each), split across
                # the two HWDGE queues
                y0 = 32 * band
                for g in range(G):
                    ENG[g // 2].dma_start(
                        x4[32 * g:32 * (g + 1), s0 * PADW:s1 * PADW]
                        .rearrange("p (s c) -> p s c", c=PADW),
                        xin_ap[b, :, y0 + g + s0:y0 + g + s1, :])

            def load_band(b, band):
                x4 = xp.tile([128, SROWS * PADW + 16], BF16, tag="x4")
                load_piece(x4, b, band, 0, SROWS)
                return x4

            def wt_tile(b):
                return wp.tile([128, NBLK * 128], BF16, tag="wt",
                               name=f"wt{b}")

            def load_wt_chunk(eng, wt, b, j, kx0=0, kx1=NKX):
                eng.dma_start(
                    wt[:, (j * NKX + kx0) * 128:(j * NKX + kx1) * 128]
                    .rearrange("k (x m) -> k x m", x=kx1 - kx0),
                    wts.ap()[b, :, j, kx0:kx1, :])

            steps = [(b, band) for b in range(BLOC) for band in range(BANDS)]

            # prologue: staircase of row pieces + weight chunks so the first
            # matmuls (pairs 0,1 of band 0) gate on the minimum bytes, and
            # later pieces land just ahead of the stream's deadlines
            wt0 = wt_tile(0)
            x4_0 = xp.tile([128, SROWS * PADW + 16], BF16, tag="x4")
            load_wt_chunk(nc.sync, wt0, 0, 0, 0, 4)
            load_piece(x4_0, 0, 0, 0, 25)
            load_wt_chunk(nc.scalar, wt0, 0, 1)
            load_wt_chunk(nc.sync, wt0, 0, 0, 4, NKX)
            load_piece(x4_0, 0, 0, 25, SROWS)
            load_wt_chunk(nc.scalar, wt0, 0, 3)
            load_wt_chunk(nc.sync, wt0, 0, 2)
            wt1 = wt_tile(1)
            for j in range(NJ):
                load_wt_chunk(ENG[j % 2], wt1, 1, j)
            wtiles = [wt0, wt1]
            x4_next = x4_0

            for si, (b, band) in enumerate(steps):
                wt = wtiles[b]
                y0 = 32 * band
                x4 = x4_next
                if si + 1 < len(steps):
                    x4_next = load_band(*steps[si + 1])
                x4a = x4[:]
                accs = [pp.tile([128, NFREE], F32, tag=f"acc{p}",
                                name=f"acc{p}_{si}")
                        for p in range(PAIRS)]
                if si == 0:
                    pair_groups = [(0, 1), (2, 3)]
                elif si == len(steps) - 1:
                    pair_groups = [(0,), (1,), (2,), (3,)]
                else:
                    pair_groups = [(0, 1, 2, 3)]
                for pg in pair_groups:
                    for j in range(NJ):
                        for kx in range(NKX):
                            wblk = wt[:, (j * NKX + kx) * 128:
                                         (j * NKX + kx + 1) * 128]
                            for p in pg:
                                s0 = 8 * p + 4 * j
                                rhs = bass.AP(
                                    x4a.tensor,
                                    x4a.offset + s0 * PADW + kx,
                                    [list(x4a.ap[0]),
                                     [4 * PADW, 2], [1, W]])
                                nc.tensor.matmul(
                                    accs[p][:], wblk, rhs,
                                    start=(j == 0 and kx == 0),
                                    stop=(j == NJ - 1 and kx == NKX - 1))
                    for p in pg:
                        ot = op.tile([128, NFREE], F32, tag="ot")
                        nc.vector.tensor_copy(ot[:], accs[p][:])
                        for q in range(2):
                            src = ot[:, q * W:(q + 1) * W]
                            dst = bass.AP(
                                yout_ap.tensor,
                                b * PD * H * W + (y0 + 8 * p + 4 * q) * W,
                                [[W, DY], [H * W, PD], [1, W]])
                            ENG[(p + q) % 2].dma_start(dst, src)
    return nc


_NC = None


def _get_nc():
    global _NC
    if _NC is None:
        _NC = _build_nc()
    return _NC


# ---------------------------------------------------------------------------
# Host side


def _gelu_exact(z):
    from math import erf
    return 0.5 * z * (1.0 + np.vectorize(erf)(z / np.sqrt(2.0)))


def _prepare_inputs(x, lk_filter, w1, b1, w2, b2):
    x = np.ascontiguousarray(np.asarray(x, dtype=np.float32))
    x1 = x[:, :PD]

    # dwc_proj on host (tiny): pool -> 1x1 -> exact GELU -> 1x1
    pooled = x1.mean(axis=(2, 3), dtype=np.float32)            # [B, 32]
    hid = _gelu_exact(pooled @ np.asarray(w1, np.float32).T
                      + np.asarray(b1, np.float32)).astype(np.float32)
    dyn_k = (hid @ np.asarray(w2, np.float32).T
             + np.asarray(b2, np.float32)).reshape(B, PD, SK, SK)

    # fold the per-sample depthwise 3x3 into the center of the 13x13 filter
    F = np.broadcast_to(np.asarray(lk_filter, np.float32),
                        (B, PD, PD, LK, LK)).copy()
    idx = np.arange(PD)
    ctr = PAD - SK // 2                                         # 5
    F[:, idx, idx, ctr:ctr + SK, ctr:ctr + SK] += dyn_k

    # weight blocks: wts[b, j, kx, g*32+ic, dy*32+oc] = F[b, oc, ic, 4j+g-dy, kx]
    wts = np.zeros((B, NJ, NKX, 128, 128), np.float32)
    for j in range(NJ):
        for g in range(G):
            for dy in range(DY):
                ky = 4 * j + g - dy
                if 0 <= ky < LK:
                    wts[:, j, :, g * PD:(g + 1) * PD,
                        dy * PD:(dy + 1) * PD] = \
                        F[:, :, :, ky, :].transpose(0, 3, 2, 1)
    # device layout [b, k, j, kx, m]: per-partition contiguous j-chunks
    wts = np.ascontiguousarray(wts.transpose(0, 3, 1, 2, 4)) \
        .astype(ml_dtypes.bfloat16)

    xpad = np.zeros((B, PD, PADH, PADW), ml_dtypes.bfloat16)
    xpad[:, :, PAD:PAD + H, PAD:PAD + W] = x1

    in_maps = [{"xin": xpad[BLOC * c:BLOC * (c + 1)],
                "wts": wts[BLOC * c:BLOC * (c + 1)]}
               for c in range(NCORES)]
    return x, in_maps


def _execute(in_maps, trace=False):
    nc = _get_nc()
    return run_bass_kernel_spmd(nc, in_maps, list(range(NCORES)), trace=trace)


def kernel(x, lk_filter, w1, b1, w2, b2):
    x, in_maps = _prepare_inputs(x, lk_filter, w1, b1, w2, b2)
    res = _execute(in_maps)
    out = np.empty((B, C, H, W), np.float32)
    for c in range(NCORES):
        out[BLOC * c:BLOC * (c + 1), :PD] = res.results[c]["yout"]
    out[:, PD:] = x[:, PD:]
    return out


# revision 11
# speedup vs baseline: 1.0049x; 1.0049x over previous
"""ConvolutionalAttention (training branch) for Trainium2, 8 NeuronCores.

The module computes, per sample b:
    out[:, :32]  = conv13x13(x1, lk_filter) + depthwise3x3(x1, dyn_k[b])
    out[:, 32:]  = x2            (pass-through)
where dyn_k[b] comes from a tiny MLP (pool -> 1x1 -> GELU -> 1x1) on x1.

Key transformation: conv is linear in the filter, so the per-sample dynamic
depthwise 3x3 kernel is folded host-side into the center of a per-sample
13x13 dense filter.  The device then runs ONE dense 32->32 13x13 conv per
sample.  Data-parallel over batch: 2 samples per core.

Device mapping (per core, per sample):
  - conv as matmul with K = 128 = (4 row-shift replicas g) x (32 in-ch),
    M = 128 = (4 output rows dy) x (32 out-ch), rhs free dim N = 384 =
    two 4-row "quads" (8 output rows) x 192 useful columns each, read via
    an overlapped access pattern into the 204-wide padded rows.
  - 52 weight blocks (4 ky'-chunks x 13 kx shifts) accumulate in PSUM.
  - bf16 operands (fp32 PSUM accumulation): full PE rate (163ns per
    N=384 matmul), half the DMA traffic of tf32, rel err ~2.5e-3.
  - single 128-partition DMAs (the DRAM AP factors the partition dim as
    [row-shift 4] x [channel 32]) reach ~245GB/s vs ~100 for per-group
    loads; weights are host-transposed so each chunk is per-partition
    contiguous.
  - prologue: pairs {0,1} run before {2,3} on the first step so compute
    starts after ~60% of the first band has landed; a PE prewarm loop
    (garbage matmuls into a scratch PSUM bank) ramps the clock to full
    speed before real work arrives.
  - last step processes pairs singly so drains/output DMAs pipeline with
    the remaining matmuls instead of serializing at the end.
"""

import json

import ml_dtypes
import numpy as np

import concourse.bass as bass
import concourse.mybir as mybir
import concourse.tile as tile
from concourse.bass_utils import run_bass_kernel_spmd

# ---------------------------------------------------------------------------
# Problem constants (hardcoded; kernel.py must be self-contained)
B, C, H, W = 16, 64, 192, 192
PD, SK, LK = 32, 3, 13
PAD = LK // 2                      # 6
NCORES = 8
BLOC = B // NCORES                 # 2 samples per core
PADW = W + 2 * PAD                 # 204
PADH = H + 2 * PAD                 # 204
NJ, G, DY = 4, 4, 4                # ky' chunks, row-shift replicas, rows/quad
NKX = LK                           # 13 kx shifts
NBLK = NJ * NKX                    # 52 weight blocks per sample
BANDS = 6                          # 32 output rows per band
PAIRS = 4                          # quad-pairs per band (8 rows each)
SROWS = 41                         # X4 rows needed per band
NFREE = 2 * W                      # 384 matmul moving free dim (2 quads)
NWARM = 56                         # prewarm matmuls (bridge to data-ready)
F32 = mybir.dt.float32
BF16 = mybir.dt.bfloat16

# ---------------------------------------------------------------------------
# Workaround: the walrus_driver in this container rejects instructions with
# more than one sync-wait command.  Post-process the BIR JSON, moving excess
# waits onto single-wait NoOps inserted right before the offending
# instruction (same engine => executes first, semantics preserved).
_orig_to_json_bytes = bass.Bass.to_json_bytes


def _split_multi_waits(m):
    for f in m.get("functions", []):
        for blk in f.get("blocks", []):
            out = []
            changed = False
            for inst in blk.get("instructions", []):
                si = inst.get("sync_info")
                waits = (si or {}).get("on_wait") or []
                if len(waits) > 1:
                    changed = True
                    for k, wcond in enumerate(waits[:-1]):
                        out.append({
                            "debug": inst.get("debug"),
                            "engine": inst["engine"],
                            "ins": [], "outs": [],
                            "name": f"{inst['name']}.sw{k}",
                            "opcode": "NoOp",
                            "sync_info": {"on_update": [], "on_wait": [wcond]},
                            "text_hint": "split_wait",
                        })
                    si["on_wait"] = [waits[-1]]
                out.append(inst)
            if changed:
                blk["instructions"] = out
    return m


def _to_json_bytes_split(self, *a, **kw):
    data = _orig_to_json_bytes(self, *a, **kw)
    return json.dumps(_split_multi_waits(json.loads(data))).encode()


_MAX_SEM = 64


def _install_patch():
    if bass.Bass.to_json_bytes is not _to_json_bytes_split:
        bass.Bass.to_json_bytes = _to_json_bytes_split
    # Shrink the semaphore space: the NEFF epilogue zeroes every semaphore
    # one instruction at a time (~8us for 255).  With --max-sem-num=64 the
    # kernel still has plenty (walrus needs ~40, bass/tile gets 64..255)
    # but the teardown loop may shrink.
    import concourse.env as _env
    if _env.get_walrus_max_sem_num() != _MAX_SEM:
        _env.get_walrus_max_sem_num = lambda: _MAX_SEM
        bass.get_walrus_max_sem_num = _env.get_walrus_max_sem_num
    import concourse.bass_utils as _bu
    if not getattr(_bu, "_sem_patched", False):
        _orig_run_command = _bu.run_command

        def _run_command_sem(cmd, *a, **kw):
            if cmd and "walrus_driver" in str(cmd[0]):
                cmd = list(cmd) + [f"--max-sem-num={_MAX_SEM}"]
            return _orig_run_command(cmd, *a, **kw)

        _bu.run_command = _run_command_sem
        _bu._sem_patched = True


# ---------------------------------------------------------------------------
# Device kernel


def _build_nc():
    _install_patch()
    nc = bass.Bass()
    xin = nc.declare_dram_parameter("xin", [BLOC, PD, PADH, PADW], BF16,
                                    isOutput=False)
    # host-transposed: [b, k(=g*32+ic), j, kx, m(=dy*32+oc)] so each j-chunk
    # is per-partition contiguous (13*128*2B = 3.3KB descriptors)
    wts = nc.declare_dram_parameter("wts", [BLOC, 128, NJ, NKX, 128], BF16,
                                    isOutput=False)
    yout = nc.declare_dram_parameter("yout", [BLOC, PD, H, W], F32,
                                     isOutput=True)
    xin_ap = xin.ap()
    yout_ap = yout.ap()

    with tile.TileContext(nc) as tc:
        with tc.tile_pool(name="wp", bufs=2) as wp, \
             tc.tile_pool(name="xp", bufs=3) as xp, \
             tc.tile_pool(name="pp", bufs=2, space="PSUM") as pp, \
             tc.tile_pool(name="op", bufs=4) as op:

            ENG = [nc.sync, nc.scalar]

            # PE prewarm: garbage matmuls into a scratch PSUM bank while the
            # prologue DMAs stream in, so HAM is at full clock (K=8/8) when
            # the real stream starts.  Reuses acc0's rotation slot.
            wi = xp.tile([128, NFREE], BF16, tag="wi")
            nc.vector.memset(wi[:], 0)
            warm = pp.tile([128, NFREE], F32, tag="acc0", name="warm")
            for i in range(NWARM):
                nc.tensor.matmul(warm[:], wi[:, :128], wi[:],
                                 start=(i == 0), stop=(i == NWARM - 1))

            def load_piece(x4, b, band, s0, s1):
                # per-g DMAs (one ascending address run each), split across
                # the two HWDGE queues
                y0 = 32 * band
                for g in range(G):
                    ENG[g // 2].dma_start(
                        x4[32 * g:32 * (g + 1), s0 * PADW:s1 * PADW]
                        .rearrange("p (s c) -> p s c", c=PADW),
                        xin_ap[b, :, y0 + g + s0:y0 + g + s1, :])

            def load_band(b, band):
                x4 = xp.tile([128, SROWS * PADW + 16], BF16, tag="x4")
                load_piece(x4, b, band, 0, SROWS)
                return x4

            def wt_tile(b):
                return wp.tile([128, NBLK * 128], BF16, tag="wt",
                               name=f"wt{b}")

            def load_wt_chunk(eng, wt, b, j, kx0=0, kx1=NKX):
                eng.dma_start(
                    wt[:, (j * NKX + kx0) * 128:(j * NKX + kx1) * 128]
                    .rearrange("k (x m) -> k x m", x=kx1 - kx0),
                    wts.ap()[b, :, j, kx0:kx1, :])

            steps = [(b, band) for b in range(BLOC) for band in range(BANDS)]

            # prologue: staircase of row pieces + weight chunks so the first
            # matmuls (pairs 0,1 of band 0) gate on the minimum bytes, and
            # later pieces land just ahead of the stream's deadlines
            wt0 = wt_tile(0)
            x4_0 = xp.tile([128, SROWS * PADW + 16], BF16, tag="x4")
            load_wt_chunk(nc.sync, wt0, 0, 0, 0, 4)
            load_piece(x4_0, 0, 0, 0, 25)
            load_wt_chunk(nc.scalar, wt0, 0, 1)
            load_wt_chunk(nc.sync, wt0, 0, 0, 4, NKX)
            load_piece(x4_0, 0, 0, 25, SROWS)
            load_wt_chunk(nc.scalar, wt0, 0, 3)
            load_wt_chunk(nc.sync, wt0, 0, 2)
            wt1 = wt_tile(1)
            for j in range(NJ):
                load_wt_chunk(ENG[j % 2], wt1, 1, j)
            wtiles = [wt0, wt1]
            x4_next = x4_0

            for si, (b, band) in enumerate(steps):
                wt = wtiles[b]
                y0 = 32 * band
                x4 = x4_next
                if si + 1 < len(steps):
                    x4_next = load_band(*steps[si + 1])
                x4a = x4[:]
                accs = [pp.tile([128, NFREE], F32, tag=f"acc{p}",
                                name=f"acc{p}_{si}")
                        for p in range(PAIRS)]
                if si == 0:
                    pair_groups = [(0, 1), (2, 3)]
                elif si == len(steps) - 1:
                    pair_groups = [(0,), (1,), (2,), (3,)]
                else:
                    pair_groups = [(0, 1, 2, 3)]
                for pg in pair_groups:
                    for j in range(NJ):
                        for kx in range(NKX):
                            wblk = wt[:, (j * NKX + kx) * 128:
                                         (j * NKX + kx + 1) * 128]
                            for p in pg:
                                s0 = 8 * p + 4 * j
                                rhs = bass.AP(
                                    x4a.tensor,
                                    x4a.offset + s0 * PADW + kx,
                                    [list(x4a.ap[0]),
                                     [4 * PADW, 2], [1, W]])
                                nc.tensor.matmul(
                                    accs[p][:], wblk, rhs,
                                    start=(j == 0 and kx == 0),
                                    stop=(j == NJ - 1 and kx == NKX - 1))
                    for p in pg:
                        ot = op.tile([128, NFREE], F32, tag="ot")
                        nc.vector.tensor_copy(ot[:], accs[p][:])
                        for q in range(2):
                            src = ot[:, q * W:(q + 1) * W]
                            dst = bass.AP(
                                yout_ap.tensor,
                                b * PD * H * W + (y0 + 8 * p + 4 * q) * W,
                                [[W, DY], [H * W, PD], [1, W]])
                            ENG[(p + q) % 2].dma_start(dst, src)
    return nc


_NC = None


def _get_nc():
    global _NC
    if _NC is None:
        _NC = _build_nc()
    return _NC


# ---------------------------------------------------------------------------
# Host side


def _gelu_exact(z):
    from math import erf
    return 0.5 * z * (1.0 + np.vectorize(erf)(z / np.sqrt(2.0)))


def _prepare_inputs(x, lk_filter, w1, b1, w2, b2):
    x = np.ascontiguousarray(np.asarray(x, dtype=np.float32))
    x1 = x[:, :PD]

    # dwc_proj on host (tiny): pool -> 1x1 -> exact GELU -> 1x1
    pooled = x1.mean(axis=(2, 3), dtype=np.float32)            # [B, 32]
    hid = _gelu_exact(pooled @ np.asarray(w1, np.float32).T
                      + np.asarray(b1, np.float32)).astype(np.float32)
    dyn_k = (hid @ np.asarray(w2, np.float32).T
             + np.asarray(b2, np.float32)).reshape(B, PD, SK, SK)

    # fold the per-sample depthwise 3x3 into the center of the 13x13 filter
    F = np.broadcast_to(np.asarray(lk_filter, np.float32),
                        (B, PD, PD, LK, LK)).copy()
    idx = np.arange(PD)
    ctr = PAD - SK // 2                                         # 5
    F[:, idx, idx, ctr:ctr + SK, ctr:ctr + SK] += dyn_k

    # weight blocks: wts[b, j, kx, g*32+ic, dy*32+oc] = F[b, oc, ic, 4j+g-dy, kx]
    wts = np.zeros((B, NJ, NKX, 128, 128), np.float32)
    for j in range(NJ):
        for g in range(G):
            for dy in range(DY):
                ky = 4 * j + g - dy
                if 0 <= ky < LK:
                    wts[:, j, :, g * PD:(g + 1) * PD,
                        dy * PD:(dy + 1) * PD] = \
                        F[:, :, :, ky, :].transpose(0, 3, 2, 1)
    # device layout [b, k, j, kx, m]: per-partition contiguous j-chunks
    wts = np.ascontiguousarray(wts.transpose(0, 3, 1, 2, 4)) \
        .astype(ml_dtypes.bfloat16)

    xpad = np.zeros((B, PD, PADH, PADW), ml_dtypes.bfloat16)
    xpad[:, :, PAD:PAD + H, PAD:PAD + W] = x1

    in_maps = [{"xin": xpad[BLOC * c:BLOC * (c + 1)],
                "wts": wts[BLOC * c:BLOC * (c + 1)]}
               for c in range(NCORES)]
    return x, in_maps


def _execute(in_maps, trace=False):
    nc = _get_nc()
    return run_bass_kernel_spmd(nc, in_maps, list(range(NCORES)), trace=trace)


def kernel(x, lk_filter, w1, b1, w2, b2):
    x, in_maps = _prepare_inputs(x, lk_filter, w1, b1, w2, b2)
    res = _execute(in_maps)
    out = np.empty((B, C, H, W), np.float32)
    for c in range(NCORES):
        out[BLOC * c:BLOC * (c + 1), :PD] = res.results[c]["yout"]
    out[:, PD:] = x[:, PD:]
    return out
